# revision 6
# baseline (speedup 1.0000x reference)
"""Trainium2 Bass kernel for a 2-layer DGCN (graph conv) on 8 NeuronCores.

Reference computation (fp32):
    h1  = relu(IFadj @ (x @ W1) + b1)         # [N, NHID]
    out = BN(adj @ (h1 @ W2) + b2)            # [N, OUTD], BN in eval mode

Distribution: rows of x / IFadj / adj are sharded across 8 cores
(row-parallel graph partitioning). Per core (rows R_k), v2 schedule:

  phase A: S_own = x[R_k] @ W1 (cb-outer over 8 PSUM banks, x/W1
           streamed in 128-row slabs so the PE starts ~1us in) ->
           bounce -> ONE S AllGather, triggered ~17us in. A tiny dummy
           collective issued at t=0 absorbs the cross-core launch
           barrier / ncfw bootstrap so the real gather starts promptly.
  phase B: redundantly compute S for global node blocks 5,6,7 (the
           same blocks on every core, from a replicated x slice) --
           this keeps the PE busy through the gather window and lets
           phase C start without waiting on the collective.
  phase C: h1T = relu(S^T @ IFadjT_k + b1), two i-half passes;
           m-traversal order 5,6,7 (local S) then 0..4 (gathered S).
           After each half: z-half = h1 @ (W2/4) evicted to fp8 and
           Z-AllGather chunk fired mid-kernel.
  phase D: outT = Z-as-lhsT vs adjT_k rhs, fp8e4 DoubleRow matmuls
           (2 m-tiles per instruction), BN fused on the PSUM evict
           with the x4 range-fold undone in the BN scale.

The PE consumes the left operand transposed (out = lhsT.T @ rhs), so
the host passes IFadj[R_k].T / adj[R_k].T per core; with the h1T/outT
formulations no on-device transposes are needed. Layer-1 matmuls run
in bf16; the layer-2 spmm runs in fp8e4 (empirically ~5e-3 rel err vs
the 2e-2 gate; adj entries are U[0,1] and z is scaled by 1/4 so both
are far inside e4m3 range).
"""

import numpy as np
import ml_dtypes

NCORES = 8
N = 8192
NFEAT = 1024
NHID = 512
OUTD = 256
ROWS = N // NCORES  # 1024
P = 128
BN_EPS = 1e-5

CB = NFEAT // P   # 8  c-blocks (x feature contraction)
IB = ROWS // P    # 8  i-blocks per node block
JB = NHID // P    # 4  j-blocks (hidden)
MT = N // P       # 64 m-tiles (global node contraction)
HF = 512          # matmul moving free dim (PSUM bank limit)
IH = ROWS // HF   # 2 i-halves of the local row range
OB = OUTD // P    # 2 output-feature blocks
GC = 2            # Z allgather chunks (one per i-half)
NRED = 3          # redundant S blocks (global blocks 5,6,7)
RED0 = NCORES - NRED  # first redundant block = 5
ORDER = list(range(RED0, NCORES)) + list(range(RED0))  # 5,6,7,0,1,2,3,4

_BF16 = ml_dtypes.bfloat16
_F8 = ml_dtypes.float8_e4m3

_cache = {}


def _build():
    import concourse.mybir as mybir
    import concourse.tile as tile
    from concourse import bacc

    dt = mybir.dt
    f32 = dt.float32
    bf16 = dt.bfloat16
    f8 = dt.float8e4
    AF = mybir.ActivationFunctionType
    DR = mybir.MatmulPerfMode.DoubleRow

    nc = bacc.Bacc("TRN2", target_bir_lowering=False, debug=False,
                   num_devices=NCORES)

    xT_e = nc.dram_tensor("xT", [NFEAT, ROWS], bf16, kind="ExternalInput")
    # replicated x rows for global node blocks 5..7 (same on every core)
    xTr_e = nc.dram_tensor("xTr", [NFEAT, NRED * ROWS], bf16,
                           kind="ExternalInput")
    ifadjT_e = nc.dram_tensor("ifadjT", [N, ROWS], bf16, kind="ExternalInput")
    adjT_e = nc.dram_tensor("adjT", [N, ROWS], f8, kind="ExternalInput")
    w1_e = nc.dram_tensor("w1", [NFEAT, NHID], bf16, kind="ExternalInput")
    w2_e = nc.dram_tensor("w2", [NHID, OUTD], bf16, kind="ExternalInput")
    b1p_e = nc.dram_tensor("b1p", [P, JB], f32, kind="ExternalInput")
    bnsc_e = nc.dram_tensor("bnsc", [P, OB], f32, kind="ExternalInput")
    bnbi_e = nc.dram_tensor("bnbi", [P, OB], f32, kind="ExternalInput")
    # outT: [OUTD, ROWS]; the host transposes each core's block.
    out_e = nc.dram_tensor("out", [OUTD, ROWS], f32, kind="ExternalOutput")

    groups = [list(range(NCORES))]

    def allgather(g_in, g_out):
        nc.gpsimd.collective_compute(
            "AllGather", mybir.AluOpType.bypass, replica_groups=groups,
            ins=[g_in[:]], outs=[g_out[:]])

    with tile.TileContext(nc) as tc:
        with (
            tc.tile_pool(name="const", bufs=1) as const,
            tc.tile_pool(name="xslab", bufs=4) as xslab_p,
            tc.tile_pool(name="sloc", bufs=1) as sloc_p,
            tc.tile_pool(name="sred", bufs=1) as sred_p,
            tc.tile_pool(name="sstage", bufs=5) as sstage_p,
            tc.tile_pool(name="h1", bufs=1) as h1_p,
            tc.tile_pool(name="zsb", bufs=1) as z_p,
            tc.tile_pool(name="zchunk", bufs=10) as zchunk_p,
            tc.tile_pool(name="astream", bufs=16) as astream,
            tc.tile_pool(name="apair", bufs=6) as apair_p,
            tc.tile_pool(name="outsb", bufs=1) as outsb_p,
            tc.tile_pool(name="dram", bufs=1, space="DRAM") as dram,
        ):
            # ---- dummy collective: absorbs the first-collective barrier
            # (cross-core launch skew + ncfw bootstrap) while the PE is
            # busy with local work; the real S gather then starts promptly.
            dmy_sb = const.tile([P, 4], f32)
            nc.gpsimd.memset(dmy_sb[:], 0.0)
            dmy_in = dram.tile([P, 4], f32, name="dmyi")
            dmy_out = dram.tile([P * NCORES, 4], f32, addr_space="Shared",
                                name="dmyo")
            nc.sync.dma_start(dmy_in[:], dmy_sb[:])
            allgather(dmy_in, dmy_out)

            # ---- constants (slab-sized DMAs so the first matmul can
            # start as soon as w1 slab 0 + xT slab 0 land)
            w1_sb = const.tile([P, CB, NHID], bf16)
            for cb in range(CB):
                nc.sync.dma_start(w1_sb[:, cb, :],
                                  w1_e[cb * P:(cb + 1) * P, :])
            b1p_sb = const.tile([P, JB], f32)
            nc.sync.dma_start(b1p_sb[:], b1p_e[:])
            w2_sb = const.tile([P, JB, OUTD], bf16)
            nc.sync.dma_start(
                w2_sb[:], w2_e[:].rearrange("(jb p) o -> p jb o", p=P))
            bnsc_sb = const.tile([P, OB], f32)
            nc.sync.dma_start(bnsc_sb[:], bnsc_e[:])
            bnbi_sb = const.tile([P, OB], f32)
            nc.sync.dma_start(bnbi_sb[:], bnbi_e[:])

            # ---- DRAM bounce buffers for the collectives
            s_bounce = dram.tile([ROWS, NHID], bf16, name="sb")
            s_all = dram.tile([N, NHID], bf16, addr_space="Shared",
                              name="sa")
            RPC = ROWS // GC  # z rows bounced per chunk (512)
            z_bounce = [dram.tile([RPC, OUTD], f8, name=f"zb{c}")
                        for c in range(GC)]
            z_all = [dram.tile([RPC * NCORES, OUTD], f8,
                               addr_space="Shared", name=f"za{c}")
                     for c in range(GC)]

            s_loc = sloc_p.tile([P, IB, NHID], bf16)
            s_red = sred_p.tile([P, NRED * IB, NHID], bf16)

            # ---- phase A: own S block, cb-outer across 8 PSUM banks;
            # bounce incrementally, fire the single S AllGather.
            with tc.tile_pool(name="psA", bufs=1, space="PSUM") as psA:
                ps_own = [psA.tile([P, NHID], f32, name=f"pso{ib}",
                                   tag=f"pa{ib}")
                          for ib in range(IB)]
                for cb in range(CB):
                    xs = xslab_p.tile([P, ROWS], bf16, tag="xslab")
                    nc.sync.dma_start(xs[:], xT_e[cb * P:(cb + 1) * P, :])
                    for ib in range(IB):
                        nc.tensor.matmul(
                            ps_own[ib][:], xs[:, ib * P:(ib + 1) * P],
                            w1_sb[:, cb, :],
                            start=(cb == 0), stop=(cb == CB - 1))
                for ib in range(IB):
                    nc.scalar.activation(s_loc[:, ib, :], ps_own[ib][:],
                                         AF.Copy)
                    nc.sync.dma_start(
                        s_bounce[ib * P:(ib + 1) * P, :], s_loc[:, ib, :])
                allgather(s_bounce, s_all)

                # ---- phase B: redundant S for global blocks 5,6,7
                for r in range(NRED):
                    ps_r = [psA.tile([P, NHID], f32, name=f"psr{r}_{ib}",
                                     tag=f"pa{ib}")
                            for ib in range(IB)]
                    for cb in range(CB):
                        xs = xslab_p.tile([P, ROWS], bf16, tag="xslab")
                        nc.sync.dma_start(
                            xs[:],
                            xTr_e[cb * P:(cb + 1) * P,
                                  r * ROWS:(r + 1) * ROWS])
                        for ib in range(IB):
                            nc.tensor.matmul(
                                ps_r[ib][:], xs[:, ib * P:(ib + 1) * P],
                                w1_sb[:, cb, :],
                                start=(cb == 0), stop=(cb == CB - 1))
                    for ib in range(IB):
                        nc.scalar.activation(
                            s_red[:, r * IB + ib, :], ps_r[ib][:], AF.Copy)

            h1T = h1_p.tile([P, JB, ROWS], bf16)
            z_sb = z_p.tile([P, IB, OUTD], f8)
            s_stage = [None] * NCORES

            # ---- phase C, i-half pass ih: accumulate h1T half over all
            # 64 m-tiles (order 5,6,7 local then 0..4 gathered), evict
            # relu half, emit z half in fp8, fire Z allgather chunk ih.
            def l1_pass(ih, psh, psz):
                psum_h = [psh.tile([P, HF], f32, name=f"ph{jb}_{ih}",
                                   tag=f"ph{jb}")
                          for jb in range(JB)]
                n_emitted = 0
                for g in ORDER:
                    if ih == 0 and g < RED0:
                        st = sstage_p.tile([P, IB, NHID], bf16,
                                           tag="sstage")
                        nc.sync.dma_start(
                            st[:],
                            s_all[g * ROWS:(g + 1) * ROWS, :]
                            .rearrange("(t p) j -> p t j", p=P))
                        s_stage[g] = st
                    for q in range(IB):
                        mt = g * IB + q
                        a_tile = astream.tile([P, HF], bf16, tag="ahalf")
                        nc.sync.dma_start(
                            a_tile[:],
                            ifadjT_e[mt * P:(mt + 1) * P,
                                     ih * HF:(ih + 1) * HF])
                        if g >= RED0:
                            s_src = s_red[:, (g - RED0) * IB + q, :]
                        else:
                            s_src = s_stage[g][:, q, :]
                        for jb in range(JB):
                            nc.tensor.matmul(
                                psum_h[jb][:],
                                s_src[:, jb * P:(jb + 1) * P],
                                a_tile[:],
                                start=(n_emitted == 0),
                                stop=(n_emitted == MT - 1),
                            )
                        n_emitted += 1
                # epilogue: relu+bias into h1T half
                for jb in range(JB):
                    nc.scalar.activation(
                        h1T[:, jb, ih * HF:(ih + 1) * HF],
                        psum_h[jb][:], AF.Relu,
                        bias=b1p_sb[:, jb:jb + 1])
                # z for this half's i-blocks (fp8, W2 pre-scaled by 1/4),
                # bounce, gather chunk ih
                for t in range(IB // IH):
                    ib = ih * (IB // IH) + t
                    ps = psz.tile([P, OUTD], f32, tag="z")
                    for jb in range(JB):
                        nc.tensor.matmul(
                            ps[:],
                            h1T[:, jb, ib * P:(ib + 1) * P],
                            w2_sb[:, jb, :],
                            start=(jb == 0), stop=(jb == JB - 1),
                        )
                    nc.scalar.activation(z_sb[:, ib, :], ps[:], AF.Copy)
                    nc.sync.dma_start(
                        z_bounce[ih][t * P:(t + 1) * P, :], z_sb[:, ib, :])
                allgather(z_bounce[ih], z_all[ih])

            with (
                tc.tile_pool(name="psh", bufs=1, space="PSUM") as psh,
                tc.tile_pool(name="psz", bufs=2, space="PSUM") as psz,
            ):
                for ih in range(IH):
                    l1_pass(ih, psh, psz)

            # ---- phase D: outT[o, i] = sum_m Z[m, o] * adjT[m, i]
            # fp8 DoubleRow: one matmul covers an adjacent m-tile pair.
            # Z-chunk c, core-block k holds m-tiles {8k + 4c + t}.
            outT_sb = outsb_p.tile([P, OB, ROWS], f32)
            QT = 4  # m-tiles per (chunk, core-block)
            with tc.tile_pool(name="ps4", bufs=1, space="PSUM") as ps4:
                psum_o = [[ps4.tile([P, HF], f32, name=f"po{ob}_{ih}",
                                    tag=f"po{ob}_{ih}")
                           for ih in range(IH)] for ob in range(OB)]
                first = True
                for c in range(GC):
                    for k in range(NCORES):
                        zc_sb = zchunk_p.tile([P, QT, OUTD], f8,
                                              tag="zchunk")
                        nc.sync.dma_start(
                            zc_sb[:],
                            z_all[c][k * QT * P:(k + 1) * QT * P, :]
                            .rearrange("(t p) o -> p t o", p=P))
                        last_grp = (c == GC - 1 and k == NCORES - 1)
                        for pr in range(0, QT, 2):
                            mt = IB * k + QT * c + pr
                            a_pair = apair_p.tile([P, 2, ROWS], f8,
                                                  tag="apair")
                            nc.sync.dma_start(
                                a_pair[:],
                                adjT_e[mt * P:(mt + 2) * P, :]
                                .rearrange("(t p) i -> p t i", p=P))
                            last_pr = last_grp and pr == QT - 2
                            for ob in range(OB):
                                for ih in range(IH):
                                    nc.tensor.matmul(
                                        psum_o[ob][ih][:],
                                        zc_sb[:, pr:pr + 2,
                                              ob * P:(ob + 1) * P],
                                        a_pair[:, :,
                                               ih * HF:(ih + 1) * HF],
                                        start=first, stop=last_pr,
                                        perf_mode=DR,
                                    )
                            first = False
                # fused BN affine on PSUM evict: out = psum*scale + bias
                for ob in range(OB):
                    for ih in range(IH):
                        nc.vector.tensor_scalar(
                            outT_sb[:, ob, ih * HF:(ih + 1) * HF],
                            psum_o[ob][ih][:],
                            bnsc_sb[:, ob:ob + 1],
                            bnbi_sb[:, ob:ob + 1],
                            mybir.AluOpType.mult,
                            mybir.AluOpType.add)
                    nc.sync.dma_start(
                        out_e[ob * P:(ob + 1) * P, :], outT_sb[:, ob, :])

    nc.compile()
    return nc


def _get_nc():
    if "nc" not in _cache:
        _cache["nc"] = _build()
    return _cache["nc"]


def kernel(x, IFadj, adj, W1, b1, W2, b2, bn_gamma, bn_beta, bn_mean, bn_var):
    from concourse.bass_utils import run_bass_kernel_spmd

    x = np.asarray(x, dtype=np.float32)
    IFadj = np.asarray(IFadj, dtype=np.float32)
    adj = np.asarray(adj, dtype=np.float32)
    W1 = np.asarray(W1, dtype=np.float32)
    b1 = np.asarray(b1, dtype=np.float32)
    W2 = np.asarray(W2, dtype=np.float32)
    b2 = np.asarray(b2, dtype=np.float32)
    bn_gamma = np.asarray(bn_gamma, dtype=np.float32)
    bn_beta = np.asarray(bn_beta, dtype=np.float32)
    bn_mean = np.asarray(bn_mean, dtype=np.float32)
    bn_var = np.asarray(bn_var, dtype=np.float32)

    # host-side prep: shard rows, transpose for PE lhsT layout, cast.
    # W2 is pre-scaled by 1/4 so z stays well inside fp8e4 range; the
    # BN scale is multiplied by 4 to undo it after the layer-2 spmm.
    w1b = W1.astype(_BF16)
    w2b = (W2 * 0.25).astype(_BF16)
    b1p = np.ascontiguousarray(b1.reshape(JB, P).T)  # [P, JB]
    inv = bn_gamma / np.sqrt(bn_var + BN_EPS)
    bias_tot = b2 * inv + bn_beta - bn_mean * inv
    bnsc = np.ascontiguousarray((4.0 * inv).reshape(OB, P).T)   # [P, OB]
    bnbi = np.ascontiguousarray(bias_tot.reshape(OB, P).T)      # [P, OB]

    # replicated x rows for global node blocks 5..7
    xTr = np.ascontiguousarray(x[RED0 * ROWS:].T).astype(_BF16)

    in_maps = []
    for k in range(NCORES):
        r0, r1 = k * ROWS, (k + 1) * ROWS
        in_maps.append({
            "xT": np.ascontiguousarray(x[r0:r1].T).astype(_BF16),
            "xTr": xTr,
            "ifadjT": np.ascontiguousarray(IFadj[r0:r1].T).astype(_BF16),
            "adjT": np.ascontiguousarray(adj[r0:r1].T).astype(_F8),
            "w1": w1b,
            "w2": w2b,
            "b1p": b1p,
            "bnsc": bnsc,
            "bnbi": bnbi,
        })

    global _last_in_maps
    _last_in_maps = in_maps

    nc = _get_nc()
    try:
        res = run_bass_kernel_spmd(nc, in_maps, list(range(NCORES)))
    except Exception:
        # transient device wedge (NRT_EXEC_UNIT_UNRECOVERABLE etc.) --
        # a straight retry has been observed to recover
        import time
        time.sleep(2.0)
        res = run_bass_kernel_spmd(nc, in_maps, list(range(NCORES)))
    # per-core output is outT [OUTD, ROWS]; transpose back and stack rows
    return np.concatenate(
        [np.ascontiguousarray(res.results[k]["out"].T)
         for k in range(NCORES)], axis=0)
